# revision 5
# baseline (speedup 1.0000x reference)
"""Fused GEMM + bias + residual + AvgPool2d(2) + global-mean normalize, 8-core SPMD.

Reference computation (B=8192, IN_F=1024, OUT_F=4096, S=64, K=2):
    out_lin = x @ W.T + bias + y                  # (B, 4096)
    pooled  = avgpool2x2(out_lin.reshape(B,64,64))# (B, 32, 32)
    out     = pooled / pooled.mean()              # (B, 1, 32, 32)

Key algebraic folds used by the kernel (all exact):
  * The 2x2 avg-pool is linear, so it folds into the weight/bias/residual:
        pooled_raw[b, m] = x[b] . Wsum[m] + bias_sum[m] + y_sum[b, m]
    where m = 32*i + j pools OUT_F rows {128i+2j, 128i+2j+1, 128i+64+2j,
    128i+64+2j+1}, and Wsum/bias_sum/y_sum are 4-row/element sums.
    This shrinks the GEMM N-dim 4096 -> 1024 (4x fewer FLOPs) and never
    materializes the (B, 4096) intermediate.
  * The 1/4 pool factor cancels between numerator and global mean:
        out = pooled_raw * (B*1024 / sum_global(pooled_raw))
  * The global sum needs one scalar AllReduce across the 8 data-parallel cores.

Sharding: batch B split 8 ways (1024 rows/core); weight + bias replicated.
"""

import numpy as np

import concourse.bass as bass
import concourse.mybir as mybir
import concourse.tile as tile
from concourse import bacc
from concourse.bass import ts
from concourse.bass_utils import run_bass_kernel_spmd
from concourse.masks import make_identity

N_CORES = 8
B = 8192
BL = B // N_CORES          # 1024 batch rows per core
KF = 1024                  # IN_F (contraction)
NF = 4096                  # OUT_F
M = 1024                   # pooled features (32*32)
TOT = float(B * M)         # elements in the global mean
F32 = mybir.dt.float32
ADD = mybir.AluOpType.add

_CACHE = {}


def build_nc():
    nc = bacc.Bacc("TRN2", target_bir_lowering=False, debug=False,
                   num_devices=N_CORES)
    x = nc.dram_tensor("x", [BL, KF], F32, kind="ExternalInput").ap()
    y = nc.dram_tensor("y", [BL, NF], F32, kind="ExternalInput").ap()
    w = nc.dram_tensor("w", [NF, KF], F32, kind="ExternalInput").ap()
    b = nc.dram_tensor("b", [1, NF], F32, kind="ExternalInput").ap()
    out = nc.dram_tensor("out", [BL, M], F32, kind="ExternalOutput").ap()

    # row index of W/bias/y-feature decomposes as n = 512g + 128a + 64r + 2j + s
    # with (g,a) -> 128g+32a+j = pooled feature m, and (r,s) the 4 pooled taps.
    wview = w.rearrange("(g a r j s) k -> g a r j s k", a=4, r=2, j=32, s=2)
    bview = b.rearrange("o (i r j s) -> o i r j s", r=2, j=32, s=2)

    with tile.TileContext(nc) as tc:
        with (
            tc.tile_pool(name="consts", bufs=1) as consts,
            tc.tile_pool(name="wload", bufs=2) as wload,
            tc.tile_pool(name="wsump", bufs=2) as wsump,
            tc.tile_pool(name="wtp", bufs=1) as wtp,
            tc.tile_pool(name="pooledp", bufs=1) as pooledp,
            tc.tile_pool(name="xload", bufs=2) as xload,
            tc.tile_pool(name="xtp", bufs=2) as xtp,
            tc.tile_pool(name="yload", bufs=2) as yload,
            tc.tile_pool(name="ysump", bufs=2) as ysump,
            tc.tile_pool(name="statsp", bufs=1) as statsp,
            tc.tile_pool(name="outp", bufs=2) as outp,
            tc.tile_pool(name="psA", bufs=4, space="PSUM") as psA,
            tc.tile_pool(name="psT", bufs=2, space="PSUM") as psT,
            tc.tile_pool(name="psS", bufs=1, space="PSUM") as psS,
            tc.tile_pool(name="dram", bufs=1, space="DRAM") as dram,
        ):
            # ---- constants ----
            ident = consts.tile([128, 128], F32)
            make_identity(nc, ident)
            ones_col = consts.tile([128, 1], F32)
            nc.vector.memset(ones_col, 1.0)
            ones_row = consts.tile([1, 128], F32)
            nc.vector.memset(ones_row, 1.0)

            # ---- bias: pool 4096 -> 1024 on one partition ----
            bl = consts.tile([1, 4, 32, 32], F32)
            for q, (r, s) in enumerate(((0, 0), (0, 1), (1, 0), (1, 1))):
                nc.sync.dma_start(out=bl[:, q, :, :], in_=bview[:, :, r, :, s])
            bsum = consts.tile([1, 32, 32], F32)
            nc.vector.tensor_add(bsum, bl[:, 0], bl[:, 1])
            nc.vector.tensor_add(bsum, bsum, bl[:, 2])
            nc.vector.tensor_add(bsum, bsum, bl[:, 3])
            bsum_f = bsum.rearrange("o i j -> o (i j)")  # [1, 1024]

            # ---- W: pool rows 4096 -> 1024, then transpose to [k, m] ----
            # wt_all[:, kb, m] holds Wsum.T for k-block kb (k on partitions).
            wt_all = wtp.tile([128, 8, M], F32)
            for g in range(8):
                wl = wload.tile([128, 4, KF], F32)
                for q, (r, s) in enumerate(((0, 0), (0, 1), (1, 0), (1, 1))):
                    nc.sync.dma_start(out=wl[:, q, :],
                                      in_=wview[g, :, r, :, s, :])
                wsum = wsump.tile([128, KF], F32)
                nc.vector.tensor_add(wsum, wl[:, 0], wl[:, 1])
                nc.vector.tensor_add(wsum, wsum, wl[:, 2])
                nc.vector.tensor_add(wsum, wsum, wl[:, 3])
                for kb in range(8):
                    pt = psT.tile([128, 128], F32)
                    nc.tensor.transpose(pt, wsum[:, ts(kb, 128)], ident)
                    nc.any.tensor_copy(out=wt_all[:, kb, ts(g, 128)], in_=pt)

            # ---- per-batch-tile: transpose x, pool y, matmul, epilogue ----
            pooled_all = pooledp.tile([128, 8, M], F32)
            psums_all = statsp.tile([128, 8], F32)
            for bt in range(8):
                xt = xload.tile([128, KF], F32)
                nc.sync.dma_start(out=xt, in_=x[ts(bt, 128), :])
                xT = xtp.tile([128, 8, 128], F32)
                for kb in range(8):
                    pt = psT.tile([128, 128], F32)
                    nc.tensor.transpose(pt, xt[:, ts(kb, 128)], ident)
                    nc.any.tensor_copy(out=xT[:, kb, :], in_=pt)

                yt = yload.tile([128, NF], F32)
                nc.sync.dma_start(out=yt, in_=y[ts(bt, 128), :])
                yv = yt.rearrange("p (i r j s) -> p i r j s", r=2, j=32, s=2)
                ys = ysump.tile([128, 32, 32], F32)
                nc.vector.tensor_add(ys, yv[:, :, 0, :, 0], yv[:, :, 0, :, 1])
                nc.vector.tensor_add(ys, ys, yv[:, :, 1, :, 0])
                nc.vector.tensor_add(ys, ys, yv[:, :, 1, :, 1])
                ys_f = ys.rearrange("p i j -> p (i j)")  # [128, 1024]

                mm = [psA.tile([128, 512], F32, tag="mm", name=f"mm{bt}_{h}")
                      for h in range(2)]
                for kb in range(8):
                    for mh in range(2):
                        nc.tensor.matmul(mm[mh], xT[:, kb, :],
                                         wt_all[:, kb, ts(mh, 512)],
                                         start=(kb == 0), stop=False)
                for mh in range(2):
                    nc.tensor.matmul(mm[mh], ones_row, bsum_f[:, ts(mh, 512)],
                                     start=False, stop=True)
                    # pooled = psum + y_sum
                    nc.vector.tensor_add(pooled_all[:, bt, ts(mh, 512)],
                                         mm[mh], ys_f[:, ts(mh, 512)])
                # row-sums for the global mean
                nc.vector.reduce_sum(out=psums_all[:, bt:bt + 1],
                                     in_=pooled_all[:, bt, :],
                                     axis=mybir.AxisListType.X)

            # ---- local scalar sum -> AllReduce -> scale = TOT / gsum ----
            colsum = statsp.tile([128, 1], F32)
            nc.vector.reduce_sum(out=colsum, in_=psums_all,
                                 axis=mybir.AxisListType.X)
            ls_ps = psS.tile([1, 1], F32, tag="small")
            nc.tensor.matmul(ls_ps, colsum, ones_col, start=True, stop=True)
            ls = statsp.tile([1, 1], F32)
            nc.vector.tensor_copy(out=ls, in_=ls_ps)

            cc_in = dram.tile([1, 1], F32)
            cc_out = dram.tile([1, 1], F32)
            nc.sync.dma_start(out=cc_in, in_=ls)
            nc.gpsimd.collective_compute(
                "AllReduce", ADD,
                replica_groups=[list(range(N_CORES))],
                ins=[cc_in.opt()], outs=[cc_out.opt()])
            gs = statsp.tile([1, 1], F32)
            nc.sync.dma_start(out=gs, in_=cc_out)

            rs = statsp.tile([1, 1], F32)
            nc.vector.reciprocal(rs, gs)
            rs2 = statsp.tile([1, 1], F32)
            nc.scalar.mul(rs2, rs, TOT)  # rs2 = TOT / gsum
            bc_ps = psS.tile([128, 1], F32, tag="small")
            nc.tensor.matmul(bc_ps, ones_row, rs2, start=True, stop=True)
            scale_b = statsp.tile([128, 1], F32)
            nc.vector.tensor_copy(out=scale_b, in_=bc_ps)

            # ---- normalize + store ----
            for bt in range(8):
                ot = outp.tile([128, M], F32)
                nc.vector.tensor_scalar_mul(ot, pooled_all[:, bt, :], scale_b)
                nc.sync.dma_start(out=out[ts(bt, 128), :], in_=ot)

    nc.compile()
    return nc


def _run(inputs, trace=False):
    if "nc" not in _CACHE:
        _CACHE["nc"] = build_nc()
    nc = _CACHE["nc"]
    x = np.ascontiguousarray(np.asarray(inputs["x"], dtype=np.float32))
    y = np.ascontiguousarray(np.asarray(inputs["y"], dtype=np.float32))
    w = np.ascontiguousarray(np.asarray(inputs["weight"], dtype=np.float32))
    b = np.ascontiguousarray(
        np.asarray(inputs["bias"], dtype=np.float32).reshape(1, NF))
    in_maps = [
        {"x": x[c * BL:(c + 1) * BL], "y": y[c * BL:(c + 1) * BL],
         "w": w, "b": b}
        for c in range(N_CORES)
    ]
    res = run_bass_kernel_spmd(nc, in_maps, core_ids=list(range(N_CORES)),
                               trace=trace)
    full = np.concatenate([res.results[c]["out"] for c in range(N_CORES)],
                          axis=0)
    return full.reshape(B, 1, 32, 32), res


def kernel(**inputs) -> np.ndarray:
    out, _ = _run(inputs, trace=False)
    return out


# revision 6
# speedup vs baseline: 1.6715x; 1.6715x over previous
"""Fused GEMM + bias + residual + AvgPool2d(2) + global-mean normalize, 8-core SPMD.

Reference computation (B=8192, IN_F=1024, OUT_F=4096, S=64, K=2):
    out_lin = x @ W.T + bias + y                  # (B, 4096)
    pooled  = avgpool2x2(out_lin.reshape(B,64,64))# (B, 32, 32)
    out     = pooled / pooled.mean()              # (B, 1, 32, 32)

Key algebraic folds used by the kernel (all exact):
  * The 2x2 avg-pool is linear, so it folds into the weight/bias/residual:
        pooled_raw[b, m] = x[b] . Wsum[m] + bias_sum[m] + y_sum[b, m]
    where m = 32*i + j pools OUT_F rows {128i+2j, 128i+2j+1, 128i+64+2j,
    128i+64+2j+1}, and Wsum/bias_sum/y_sum are 4-row/element sums.
    This shrinks the GEMM N-dim 4096 -> 1024 (4x fewer FLOPs) and never
    materializes the (B, 4096) intermediate.
  * The 1/4 pool factor cancels between numerator and global mean:
        out = pooled_raw * (B*1024 / sum_global(pooled_raw))
  * The global sum needs one scalar AllReduce across the 8 data-parallel cores.

Implementation notes (performance):
  * GEMM inputs are cast to bf16 on-chip (fp32 PSUM accumulation); all pooling
    sums and the output stay fp32.  Measured scale-relative error ~1e-4.
  * W row-pairs (2j, 2j+1) are contiguous in DRAM, so W is loaded as 8 KB
    descriptors with a 32-wide outer partition dim to spread across all 16
    SDMA engines.  Loads alternate between the two HWDGE rings (sync/scalar);
    x is loaded with a casting SWDGE DMA on gpsimd.
  * x is loaded + transposed first so the PE has work while W streams in.

Sharding: batch B split 8 ways (1024 rows/core); weight + bias replicated.
"""

import numpy as np

import concourse.bass as bass
import concourse.mybir as mybir
import concourse.tile as tile
from concourse import bacc
from concourse.bass import ts
from concourse.bass_utils import run_bass_kernel_spmd
from concourse.masks import make_identity

N_CORES = 8
B = 8192
BL = B // N_CORES          # 1024 batch rows per core
KF = 1024                  # IN_F (contraction)
NF = 4096                  # OUT_F
M = 1024                   # pooled features (32*32)
TOT = float(B * M)         # elements in the global mean
F32 = mybir.dt.float32
BF16 = mybir.dt.bfloat16
ADD = mybir.AluOpType.add

_CACHE = {}


def build_nc():
    nc = bacc.Bacc("TRN2", target_bir_lowering=False, debug=False,
                   num_devices=N_CORES)
    x = nc.dram_tensor("x", [BL, KF], F32, kind="ExternalInput").ap()
    y = nc.dram_tensor("y", [BL, NF], F32, kind="ExternalInput").ap()
    w = nc.dram_tensor("w", [NF, KF], F32, kind="ExternalInput").ap()
    b = nc.dram_tensor("b", [1, NF], F32, kind="ExternalInput").ap()
    out = nc.dram_tensor("out", [BL, M], F32, kind="ExternalOutput").ap()

    # W row index decomposes as n = 512g + 128a + 64r + 2j + s; the pooled
    # feature is m = 128g + 32a + j and (r, s) are the 4 pooled taps.  Rows
    # (2j, 2j+1) are adjacent, so pair them into one contiguous 2048-elem
    # (8 KB) chunk: row-pair index np = 256g + 64a + 32r + j.
    w_pairs = w.rearrange("(n s) k -> n (s k)", s=2)          # [2048, 2048]
    wv = w_pairs.rearrange("(g a r j) kk -> g r a j kk", a=4, r=2, j=32)
    bview = b.rearrange("o (i r j s) -> o i r j s", r=2, j=32, s=2)

    # alternate big loads across the two HWDGE rings
    ring = [nc.sync, nc.scalar]

    with tile.TileContext(nc) as tc:
        with (
            tc.tile_pool(name="consts", bufs=1) as consts,
            tc.tile_pool(name="wload", bufs=2) as wload,
            tc.tile_pool(name="wsump", bufs=2) as wsump,
            tc.tile_pool(name="wtp", bufs=1) as wtp,
            tc.tile_pool(name="pooledp", bufs=1) as pooledp,
            tc.tile_pool(name="xload", bufs=2) as xload,
            tc.tile_pool(name="xtp", bufs=1) as xtp,
            tc.tile_pool(name="yload", bufs=3) as yload,
            tc.tile_pool(name="yup", bufs=2) as yup,
            tc.tile_pool(name="ysump", bufs=4) as ysump,
            tc.tile_pool(name="statsp", bufs=1) as statsp,
            tc.tile_pool(name="outp", bufs=2) as outp,
            tc.tile_pool(name="psA", bufs=4, space="PSUM") as psA,
            tc.tile_pool(name="psT", bufs=2, space="PSUM") as psT,
            tc.tile_pool(name="psS", bufs=1, space="PSUM") as psS,
            tc.tile_pool(name="dram", bufs=1, space="DRAM") as dram,
        ):
            # ---- constants ----
            ident = consts.tile([128, 128], BF16)
            make_identity(nc, ident)
            ones_row = consts.tile([1, 128], BF16)
            nc.vector.memset(ones_row, 1.0)
            ones_col = consts.tile([128, 1], F32)
            nc.vector.memset(ones_col, 1.0)

            # ---- bias: pool 4096 -> 1024 on one partition, cast to bf16 ----
            bload = consts.tile([1, 4, 32, 32], F32)
            for q, (r, s) in enumerate(((0, 0), (0, 1), (1, 0), (1, 1))):
                nc.gpsimd.dma_start(out=bload[:, q, :, :],
                                    in_=bview[:, :, r, :, s])
            bsum = consts.tile([1, 32, 32], F32)
            nc.vector.tensor_add(bsum, bload[:, 0], bload[:, 1])
            nc.vector.tensor_add(bsum, bsum, bload[:, 2])
            nc.vector.tensor_add(bsum, bsum, bload[:, 3])
            bsum_bf = consts.tile([1, M], BF16)
            nc.vector.tensor_copy(out=bsum_bf,
                                  in_=bsum.rearrange("o i j -> o (i j)"))

            # ---- x: casting load (fp32->bf16) + PE transpose, all b-tiles ----
            xT = xtp.tile([128, 8, 8, 128], BF16)   # [k, bt, kb, b]
            for bt in range(8):
                xbf = xload.tile([128, KF], BF16)
                nc.gpsimd.dma_start(out=xbf, in_=x[ts(bt, 128), :])
                for kb in range(8):
                    pt = psT.tile([128, 128], BF16)
                    nc.tensor.transpose(pt, xbf[:, ts(kb, 128)], ident)
                    nc.vector.tensor_copy(out=xT[:, bt, kb, :], in_=pt)

            # ---- W: pool rows 4096 -> 1024 (bf16), transpose to [k, m] ----
            wt_all = wtp.tile([128, 8, M], BF16)
            for g in range(8):
                wl = wload.tile([128, 2, 2048], F32)
                eng = ring[g % 2]
                for r in range(2):
                    for a in range(4):
                        eng.dma_start(out=wl[ts(a, 32), r, :],
                                      in_=wv[g, r, a])
                wlv = wl.rearrange("p r (s k) -> p r s k", s=2)
                t1 = wsump.tile([128, KF], F32)
                nc.vector.tensor_add(t1, wlv[:, 0, 0], wlv[:, 0, 1])
                t2 = wsump.tile([128, KF], F32)
                nc.vector.tensor_add(t2, wlv[:, 1, 0], wlv[:, 1, 1])
                wsum = wsump.tile([128, KF], BF16)
                nc.vector.tensor_add(wsum, t1, t2)
                for kb in range(8):
                    pt = psT.tile([128, 128], BF16)
                    nc.tensor.transpose(pt, wsum[:, ts(kb, 128)], ident)
                    nc.scalar.copy(out=wt_all[:, kb, ts(g, 128)], in_=pt)

            # ---- y: stream halves, pool 2048 -> 512 each ----
            # y half nh of b-tile bt covers pooled features [512*nh, 512*nh+512)
            ys_tiles = {}
            for bt in range(8):
                for nh in range(2):
                    yt = yload.tile([128, 2048], F32)
                    ring[(2 * bt + nh) % 2].dma_start(
                        out=yt, in_=y[ts(bt, 128), ts(nh, 2048)])
                    ytv = yt.rearrange("p (q s) -> p q s", s=2)
                    u = yup.tile([128, KF], F32)
                    nc.vector.tensor_add(u, ytv[:, :, 0], ytv[:, :, 1])
                    u2 = u.rearrange("p (i r j) -> p i r j", r=2, j=32)
                    ys = ysump.tile([128, 512], F32, tag="ys",
                                    name=f"ys{bt}_{nh}")
                    ysv = ys.rearrange("p (i j) -> p i j", j=32)
                    nc.vector.tensor_add(ysv, u2[:, :, 0, :], u2[:, :, 1, :])
                    ys_tiles[(bt, nh)] = ys

            # ---- GEMM + epilogue per b-tile ----
            pooled_all = pooledp.tile([128, 8, M], F32)
            psums_all = statsp.tile([128, 8], F32)
            for bt in range(8):
                mm = [psA.tile([128, 512], F32, tag="mm", name=f"mm{bt}_{h}")
                      for h in range(2)]
                for kb in range(8):
                    for mh in range(2):
                        nc.tensor.matmul(mm[mh], xT[:, bt, kb, :],
                                         wt_all[:, kb, ts(mh, 512)],
                                         start=(kb == 0), stop=False)
                for mh in range(2):
                    nc.tensor.matmul(mm[mh], ones_row, bsum_bf[:, ts(mh, 512)],
                                     start=False, stop=True)
                    nc.vector.tensor_add(pooled_all[:, bt, ts(mh, 512)],
                                         mm[mh], ys_tiles[(bt, mh)])
                nc.vector.reduce_sum(out=psums_all[:, bt:bt + 1],
                                     in_=pooled_all[:, bt, :],
                                     axis=mybir.AxisListType.X)

            # ---- local scalar sum -> AllReduce -> scale = TOT / gsum ----
            colsum = statsp.tile([128, 1], F32)
            nc.vector.reduce_sum(out=colsum, in_=psums_all,
                                 axis=mybir.AxisListType.X)
            ls_ps = psS.tile([1, 1], F32, tag="small")
            nc.tensor.matmul(ls_ps, colsum, ones_col, start=True, stop=True)
            ls = statsp.tile([1, 1], F32)
            nc.vector.tensor_copy(out=ls, in_=ls_ps)

            cc_in = dram.tile([1, 1], F32)
            cc_out = dram.tile([1, 1], F32)
            nc.sync.dma_start(out=cc_in, in_=ls)
            nc.gpsimd.collective_compute(
                "AllReduce", ADD,
                replica_groups=[list(range(N_CORES))],
                ins=[cc_in.opt()], outs=[cc_out.opt()])
            gs = statsp.tile([1, 1], F32)
            nc.sync.dma_start(out=gs, in_=cc_out)

            rs = statsp.tile([1, 1], F32)
            nc.vector.reciprocal(rs, gs)
            rs2 = statsp.tile([1, 1], F32)
            nc.scalar.mul(rs2, rs, TOT)  # rs2 = TOT / gsum
            ones_row_f = statsp.tile([1, 128], F32)
            nc.vector.memset(ones_row_f, 1.0)
            bc_ps = psS.tile([128, 1], F32, tag="small")
            nc.tensor.matmul(bc_ps, ones_row_f, rs2, start=True, stop=True)
            scale_b = statsp.tile([128, 1], F32)
            nc.vector.tensor_copy(out=scale_b, in_=bc_ps)

            # ---- normalize + store ----
            for bt in range(8):
                ot = outp.tile([128, M], F32)
                nc.vector.tensor_scalar_mul(ot, pooled_all[:, bt, :], scale_b)
                ring[bt % 2].dma_start(out=out[ts(bt, 128), :], in_=ot)

    nc.compile()
    return nc


def _run(inputs, trace=False):
    if "nc" not in _CACHE:
        _CACHE["nc"] = build_nc()
    nc = _CACHE["nc"]
    x = np.ascontiguousarray(np.asarray(inputs["x"], dtype=np.float32))
    y = np.ascontiguousarray(np.asarray(inputs["y"], dtype=np.float32))
    w = np.ascontiguousarray(np.asarray(inputs["weight"], dtype=np.float32))
    b = np.ascontiguousarray(
        np.asarray(inputs["bias"], dtype=np.float32).reshape(1, NF))
    in_maps = [
        {"x": x[c * BL:(c + 1) * BL], "y": y[c * BL:(c + 1) * BL],
         "w": w, "b": b}
        for c in range(N_CORES)
    ]
    res = run_bass_kernel_spmd(nc, in_maps, core_ids=list(range(N_CORES)),
                               trace=trace)
    full = np.concatenate([res.results[c]["out"] for c in range(N_CORES)],
                          axis=0)
    return full.reshape(B, 1, 32, 32), res


def kernel(**inputs) -> np.ndarray:
    out, _ = _run(inputs, trace=False)
    return out


# revision 8
# speedup vs baseline: 1.7587x; 1.0522x over previous
"""Fused GEMM + bias + residual + AvgPool2d(2) + global-mean normalize, 8-core SPMD.

Reference computation (B=8192, IN_F=1024, OUT_F=4096, S=64, K=2):
    out_lin = x @ W.T + bias + y                  # (B, 4096)
    pooled  = avgpool2x2(out_lin.reshape(B,64,64))# (B, 32, 32)
    out     = pooled / pooled.mean()              # (B, 1, 32, 32)

Key algebraic folds used by the kernel (all exact):
  * The 2x2 avg-pool is linear, so it folds into the weight/bias/residual:
        pooled_raw[b, m] = x[b] . Wsum[m] + bias_sum[m] + y_sum[b, m]
    where m = 32*i + j pools OUT_F rows {128i+2j, 128i+2j+1, 128i+64+2j,
    128i+64+2j+1}, and Wsum/bias_sum/y_sum are 4-row/element sums.
    This shrinks the GEMM N-dim 4096 -> 1024 (4x fewer FLOPs) and never
    materializes the (B, 4096) intermediate.
  * The 1/4 pool factor cancels between numerator and global mean:
        out = pooled_raw * (B*1024 / sum_global(pooled_raw))
  * The global sum needs one scalar AllReduce across the 8 data-parallel cores.

Implementation notes (performance):
  * GEMM inputs are cast to bf16 on-chip (fp32 PSUM accumulation); all pooling
    sums and the output stay fp32.  Measured scale-relative error ~1e-4.
  * W row-pairs (2j, 2j+1) are contiguous in DRAM, so W is loaded as 8 KB
    descriptors with a 32-wide outer partition dim to spread across all 16
    SDMA engines.  Loads alternate between the two HWDGE rings (sync/scalar);
    x is loaded with a casting SWDGE DMA on gpsimd.
  * x is loaded + transposed first so the PE has work while W streams in.

Sharding: batch B split 8 ways (1024 rows/core); weight + bias replicated.
"""

import numpy as np

import concourse.bass as bass
import concourse.mybir as mybir
import concourse.tile as tile
from concourse import bacc
from concourse.bass import ts
from concourse.bass_utils import run_bass_kernel_spmd
from concourse.masks import make_identity

N_CORES = 8
B = 8192
BL = B // N_CORES          # 1024 batch rows per core
KF = 1024                  # IN_F (contraction)
NF = 4096                  # OUT_F
M = 1024                   # pooled features (32*32)
TOT = float(B * M)         # elements in the global mean
F32 = mybir.dt.float32
BF16 = mybir.dt.bfloat16
ADD = mybir.AluOpType.add

_CACHE = {}


def build_nc():
    nc = bacc.Bacc("TRN2", target_bir_lowering=False, debug=False,
                   num_devices=N_CORES)
    x = nc.dram_tensor("x", [BL, KF], F32, kind="ExternalInput").ap()
    y = nc.dram_tensor("y", [BL, NF], F32, kind="ExternalInput").ap()
    w = nc.dram_tensor("w", [NF, KF], F32, kind="ExternalInput").ap()
    b = nc.dram_tensor("b", [1, NF], F32, kind="ExternalInput").ap()
    out = nc.dram_tensor("out", [BL, M], F32, kind="ExternalOutput").ap()

    # W row index decomposes as n = 512g + 128a + 64r + 2j + s; the pooled
    # feature is m = 128g + 32a + j and (r, s) are the 4 pooled taps.  Rows
    # (2j, 2j+1) are adjacent, so pair them into one contiguous 2048-elem
    # (8 KB) chunk: row-pair index np = 256g + 64a + 32r + j.
    w_pairs = w.rearrange("(n s) k -> n (s k)", s=2)          # [2048, 2048]
    wv = w_pairs.rearrange("(g a r j) kk -> g r a j kk", a=4, r=2, j=32)
    bview = b.rearrange("o (i r j s) -> o i r j s", r=2, j=32, s=2)

    # alternate big loads across the two HWDGE rings
    ring = [nc.sync, nc.scalar]

    with tile.TileContext(nc) as tc:
        with (
            tc.tile_pool(name="consts", bufs=1) as consts,
            tc.tile_pool(name="wload", bufs=2) as wload,
            tc.tile_pool(name="wsump", bufs=1) as wsump,
            tc.tile_pool(name="wtp", bufs=1) as wtp,
            tc.tile_pool(name="pooledp", bufs=1) as pooledp,
            tc.tile_pool(name="xload", bufs=2) as xload,
            tc.tile_pool(name="xtp", bufs=1) as xtp,
            tc.tile_pool(name="yload", bufs=2) as yload,
            tc.tile_pool(name="xbfp", bufs=2) as xbfp,
            tc.tile_pool(name="yup", bufs=2) as yup,
            tc.tile_pool(name="ysump", bufs=1) as ysump,
            tc.tile_pool(name="statsp", bufs=1) as statsp,
            tc.tile_pool(name="outp", bufs=2) as outp,
            tc.tile_pool(name="psA", bufs=4, space="PSUM") as psA,
            tc.tile_pool(name="psT", bufs=2, space="PSUM") as psT,
            tc.tile_pool(name="psS", bufs=1, space="PSUM") as psS,
            tc.tile_pool(name="dram", bufs=1, space="DRAM") as dram,
        ):
            # ---- constants ----
            ident = consts.tile([128, 128], BF16)
            make_identity(nc, ident)
            ones_row = consts.tile([1, 128], BF16)
            nc.vector.memset(ones_row, 1.0)
            ones_col = consts.tile([128, 1], F32)
            nc.vector.memset(ones_col, 1.0)

            # ---- bias: pool 4096 -> 1024 via accumulating SWDGE DMAs ----
            bsum = consts.tile([1, 32, 32], F32)
            for q, (r, s) in enumerate(((0, 0), (0, 1), (1, 0), (1, 1))):
                nc.gpsimd.dma_start(
                    out=bsum, in_=bview[:, :, r, :, s],
                    accum_op=(ADD if q else mybir.AluOpType.bypass))
            bsum_bf = consts.tile([1, M], BF16)
            nc.vector.tensor_copy(out=bsum_bf,
                                  in_=bsum.rearrange("o i j -> o (i j)"))

            # ---- x: HWDGE fp32 load, ACT cast to bf16, PE transpose ----
            xT = xtp.tile([128, 8, 8, 128], BF16)   # [k, bt, kb, b]
            for bt in range(8):
                xf = xload.tile([128, KF], F32)
                ring[bt % 2].dma_start(out=xf, in_=x[ts(bt, 128), :])
                xbf = xbfp.tile([128, KF], BF16)
                nc.scalar.copy(out=xbf, in_=xf)
                for kb in range(8):
                    pt = psT.tile([128, 128], BF16)
                    nc.tensor.transpose(pt, xbf[:, ts(kb, 128)], ident)
                    nc.vector.tensor_copy(out=xT[:, bt, kb, :], in_=pt)

            # ---- W: pool rows 4096 -> 1024 (bf16), transpose to [k, m] ----
            wt_all = wtp.tile([128, 8, M], BF16)
            for g in range(8):
                wl = wload.tile([128, 2, 2048], F32)
                eng = ring[g % 2]
                for r in range(2):
                    for a in range(4):
                        eng.dma_start(out=wl[ts(a, 32), r, :],
                                      in_=wv[g, r, a])
                wlv = wl.rearrange("p r (s k) -> p r s k", s=2)
                t1 = wsump.tile([128, KF], F32)
                nc.vector.tensor_add(t1, wlv[:, 0, 0], wlv[:, 0, 1])
                t2 = wsump.tile([128, KF], F32)
                nc.vector.tensor_add(t2, wlv[:, 1, 0], wlv[:, 1, 1])
                wsum = wsump.tile([128, KF], BF16)
                nc.vector.tensor_add(wsum, t1, t2)
                for kb in range(8):
                    pt = psT.tile([128, 128], BF16)
                    nc.tensor.transpose(pt, wsum[:, ts(kb, 128)], ident)
                    nc.scalar.copy(out=wt_all[:, kb, ts(g, 128)], in_=pt)

            # ---- y: stream halves, pool 2048 -> 512 each ----
            # y half nh of b-tile bt covers pooled features [512*nh, 512*nh+512)
            ys_tiles = {}
            for bt in range(8):
                for nh in range(2):
                    yt = yload.tile([128, 2048], F32)
                    ring[(2 * bt + nh) % 2].dma_start(
                        out=yt, in_=y[ts(bt, 128), ts(nh, 2048)])
                    ytv = yt.rearrange("p (q s) -> p q s", s=2)
                    u = yup.tile([128, KF], F32)
                    nc.vector.tensor_add(u, ytv[:, :, 0], ytv[:, :, 1])
                    u2 = u.rearrange("p (i r j) -> p i r j", r=2, j=32)
                    ys = ysump.tile([128, 512], F32, tag=f"ys{bt}_{nh}",
                                    name=f"ys{bt}_{nh}")
                    ysv = ys.rearrange("p (i j) -> p i j", j=32)
                    nc.vector.tensor_add(ysv, u2[:, :, 0, :], u2[:, :, 1, :])
                    ys_tiles[(bt, nh)] = ys

            # ---- GEMM + epilogue per b-tile ----
            pooled_all = pooledp.tile([128, 8, M], F32)
            psums_all = statsp.tile([128, 8], F32)
            for bt in range(8):
                mm = [psA.tile([128, 512], F32, tag="mm", name=f"mm{bt}_{h}")
                      for h in range(2)]
                for kb in range(8):
                    for mh in range(2):
                        nc.tensor.matmul(mm[mh], xT[:, bt, kb, :],
                                         wt_all[:, kb, ts(mh, 512)],
                                         start=(kb == 0), stop=False)
                for mh in range(2):
                    nc.tensor.matmul(mm[mh], ones_row, bsum_bf[:, ts(mh, 512)],
                                     start=False, stop=True)
                    nc.vector.tensor_add(pooled_all[:, bt, ts(mh, 512)],
                                         mm[mh], ys_tiles[(bt, mh)])
                nc.vector.reduce_sum(out=psums_all[:, bt:bt + 1],
                                     in_=pooled_all[:, bt, :],
                                     axis=mybir.AxisListType.X)

            # ---- local scalar sum -> AllReduce -> scale = TOT / gsum ----
            colsum = statsp.tile([128, 1], F32)
            nc.vector.reduce_sum(out=colsum, in_=psums_all,
                                 axis=mybir.AxisListType.X)
            ls_ps = psS.tile([1, 1], F32, tag="small")
            nc.tensor.matmul(ls_ps, colsum, ones_col, start=True, stop=True)
            ls = statsp.tile([1, 1], F32)
            nc.vector.tensor_copy(out=ls, in_=ls_ps)

            cc_in = dram.tile([1, 1], F32)
            cc_out = dram.tile([1, 1], F32)
            nc.sync.dma_start(out=cc_in, in_=ls)
            nc.gpsimd.collective_compute(
                "AllReduce", ADD,
                replica_groups=[list(range(N_CORES))],
                ins=[cc_in.opt()], outs=[cc_out.opt()])
            gs = statsp.tile([1, 1], F32)
            nc.sync.dma_start(out=gs, in_=cc_out)

            rs = statsp.tile([1, 1], F32)
            nc.vector.reciprocal(rs, gs)
            rs2 = statsp.tile([1, 1], F32)
            nc.scalar.mul(rs2, rs, TOT)  # rs2 = TOT / gsum
            ones_row_f = statsp.tile([1, 128], F32)
            nc.vector.memset(ones_row_f, 1.0)
            bc_ps = psS.tile([128, 1], F32, tag="small")
            nc.tensor.matmul(bc_ps, ones_row_f, rs2, start=True, stop=True)
            scale_b = statsp.tile([128, 1], F32)
            nc.vector.tensor_copy(out=scale_b, in_=bc_ps)

            # ---- normalize + store ----
            for bt in range(8):
                ot = outp.tile([128, M], F32)
                nc.vector.tensor_scalar_mul(ot, pooled_all[:, bt, :], scale_b)
                ring[bt % 2].dma_start(out=out[ts(bt, 128), :], in_=ot)

    nc.compile()
    return nc


def _run(inputs, trace=False):
    if "nc" not in _CACHE:
        _CACHE["nc"] = build_nc()
    nc = _CACHE["nc"]
    x = np.ascontiguousarray(np.asarray(inputs["x"], dtype=np.float32))
    y = np.ascontiguousarray(np.asarray(inputs["y"], dtype=np.float32))
    w = np.ascontiguousarray(np.asarray(inputs["weight"], dtype=np.float32))
    b = np.ascontiguousarray(
        np.asarray(inputs["bias"], dtype=np.float32).reshape(1, NF))
    in_maps = [
        {"x": x[c * BL:(c + 1) * BL], "y": y[c * BL:(c + 1) * BL],
         "w": w, "b": b}
        for c in range(N_CORES)
    ]
    res = run_bass_kernel_spmd(nc, in_maps, core_ids=list(range(N_CORES)),
                               trace=trace)
    full = np.concatenate([res.results[c]["out"] for c in range(N_CORES)],
                          axis=0)
    return full.reshape(B, 1, 32, 32), res


def kernel(**inputs) -> np.ndarray:
    out, _ = _run(inputs, trace=False)
    return out
